# revision 1
# baseline (speedup 1.0000x reference)
"""Trainium2 Bass kernel for DipolePredictorE3NN.

Reference computation (per batch b of B=1024, over N=4096 nodes):
    s      = feats[..., :5] @ w_path0                      # scalar gate
    tp     = C01 * s * edge + C11*w_path1[0] * cross(feats[...,5:8], edge)
    g      = tp.mean(nodes)                                # [B, 3]
    out    = relu(g @ W1 + b1) @ W2 + b2                   # [B, 3]

Strategy: data-parallel over batch, 8 cores x 128 batches; partition dim
= local batch (exactly 128), free dim = nodes. Inputs are cast to bf16
on the host (output tolerance is 2e-2; bf16 contributes ~4e-3): halves
HBM traffic AND unlocks the DVE 2x/4x packed perf modes, which only
built-in ops with 2-byte packed SBUF operands hit. Fused mult-reduce
ops (AMR/STT/TTR) are locked to 1x, so the 13x 1x-pass baseline loses
to a split schedule: products at 2x, reductions elsewhere.

Per node-tile (bf16 planes):
  - DVE gate: s' = sum_u (w_u*C01/N) x_u as 5 tensor_scalar (4x) +
    3 concatenated tensor_tensor adds (2x).
  - DVE bakes the cross coefficient into the operands: vpos = v*(+c2),
    vneg = v*(-c2), two 4x tensor_scalar ops over the 3-plane concat.
  - DVE products (2x tensor_tensor) write k-major planes
    [s_k | vpos_{k+1}*e_{k+2} | vneg_{k+2}*e_{k+1}] so each k's three
    terms are CONTIGUOUS with a uniform coefficient of 1.0; the
    s'-products are one strided broadcast-s' op, the crosses 6 ops.
  - Reductions: ACT (otherwise idle) reduces each k-triple with ONE
    activation(Copy, accum_out) over [128, 3T] - 3 ops per tile
    instead of 9, because every accum op costs a separate ~334 ns
    ACTIVATION_READ_ACCUMULATOR on top of ~440 ns issue overhead.
    Measured: every fused mult-reduce path (AMR/STT-accum/ts-accum
    aka TENSOR_SCALAR_CACHE_REDUCE) runs at ~1x, so products at 2x +
    coarse ACT accumulation beats any fused form.
  - Tail: one strided tensor_reduce folds partials into g [128, 3];
    PE transpose + 2 matmuls run the tiny MLP: hT = relu(W1b^T.[g|1]T),
    outT = W2^T.hT + b2. Host concatenates per-core outT [3, 128].
"""

import sys

if "/opt/trn_rl_repo" not in sys.path:
    sys.path.insert(0, "/opt/trn_rl_repo")

import numpy as np

try:
    import ml_dtypes

    BF16 = np.dtype(ml_dtypes.bfloat16)
except ImportError:  # pragma: no cover
    BF16 = np.dtype("bfloat16")

C01 = float(np.sqrt(0.5) / np.sqrt(3.0))
C11 = float(np.sqrt(0.5) / np.sqrt(6.0))

B, N = 1024, 4096
NCORES = 8
BL = B // NCORES  # 128 local batches = partition count

# ramp-up tile sizes
TILES = [512, 1024, 1280, 1280]
assert sum(TILES) == N
TMAX = max(TILES)

# feats channel order on the wire (natural)
F_ORDER = [0, 1, 2, 3, 4, 5, 6, 7]
# edge channel order on the wire (natural)
E_ORDER = [0, 1, 2]

FIO_BUFS = 3
EIO_BUFS = 3

_CACHED = {}


def _build(w0_vals, w1_val):
    import concourse.bacc as bacc
    import concourse.mybir as mybir
    from concourse import tile
    from concourse.masks import make_identity

    f32 = mybir.dt.float32
    bf16 = mybir.dt.bfloat16
    Alu = mybir.AluOpType
    Act = mybir.ActivationFunctionType

    # gate immediates carry weight * C01 / N -> s-term reduce scale is 1.0
    w0s = [float(w) * C01 / float(N) for w in w0_vals]
    c2 = float(w1_val) * C11 / float(N)  # cross-product coefficient

    nc = bacc.Bacc("TRN2", debug=False)

    feats = nc.dram_tensor("feats", [BL, 8 * N], bf16, kind="ExternalInput").ap()
    edge = nc.dram_tensor("edge", [BL, 3 * N], bf16, kind="ExternalInput").ap()
    W1 = nc.dram_tensor("W1", [3, 128], f32, kind="ExternalInput").ap()
    b1 = nc.dram_tensor("b1", [1, 128], f32, kind="ExternalInput").ap()
    W2 = nc.dram_tensor("W2", [128, 3], f32, kind="ExternalInput").ap()
    b2 = nc.dram_tensor("b2", [3, 1], f32, kind="ExternalInput").ap()
    outT = nc.dram_tensor("outT", [3, BL], f32, kind="ExternalOutput").ap()

    NT_ = len(TILES)

    with tile.TileContext(nc) as tc:
        with (
            tc.tile_pool(name="consts", bufs=1) as consts,
            tc.tile_pool(name="state", bufs=1) as state,
            tc.tile_pool(name="fio", bufs=FIO_BUFS) as fio,
            tc.tile_pool(name="eio", bufs=EIO_BUFS) as eio,
            tc.tile_pool(name="sw", bufs=1) as sw,
            tc.tile_pool(name="prod", bufs=2) as prodp,
            tc.tile_pool(name="psum", bufs=1, space="PSUM") as psum,
        ):
            # first tile's streams before anything else
            Tt0 = TILES[0]
            ftile0 = fio.tile([128, 8 * TMAX], bf16, tag="f", name="ftile0")
            nc.sync.dma_start(out=ftile0[:, : 8 * Tt0], in_=feats[:, : 8 * Tt0])
            etile0 = eio.tile([128, 3 * TMAX], bf16, tag="e", name="etile0")
            nc.sync.dma_start(out=etile0[:, : 3 * Tt0], in_=edge[:, : 3 * Tt0])

            # constants + PE-side setup up front
            identity = consts.tile([128, 128], f32)
            make_identity(nc, identity[:])
            w1b_s = consts.tile([4, 128], f32)
            nc.sync.dma_start(out=w1b_s[0:3, :], in_=W1)
            nc.sync.dma_start(out=w1b_s[3:4, :], in_=b1)
            w2_s = consts.tile([128, 3], f32)
            nc.sync.dma_start(out=w2_s[:], in_=W2)
            b2_s = consts.tile([3, 1], f32)
            nc.sync.dma_start(out=b2_s[:], in_=b2)

            # acc[:, 0:3] holds g; col 3 = 1.0 feeds the bias fold
            acc = state.tile([128, 4], f32)
            nc.vector.memset(acc[:, 3:4], 1.0)

            # per-(tile, k) partials: col = t*3 + k
            pcol = state.tile([128, NT_ * 3], f32)

            # ACT reduce passes dump their elementwise out here
            dump_a = state.tile([128, 3 * TMAX], bf16)

            foff = 0
            eoff = 0
            for t, Tt in enumerate(TILES):
                if t == 0:
                    ftile, etile = ftile0, etile0
                else:
                    ftile = fio.tile([128, 8 * TMAX], bf16, tag="f", name="ftile")
                    nc.sync.dma_start(
                        out=ftile[:, : 8 * Tt], in_=feats[:, foff : foff + 8 * Tt]
                    )
                    etile = eio.tile([128, 3 * TMAX], bf16, tag="e", name="etile")
                    nc.sync.dma_start(
                        out=etile[:, : 3 * Tt], in_=edge[:, eoff : eoff + 3 * Tt]
                    )
                foff += 8 * Tt
                eoff += 3 * Tt

                x = [ftile[:, u * Tt : (u + 1) * Tt] for u in range(5)]
                v = [ftile[:, (5 + u) * Tt : (6 + u) * Tt] for u in range(3)]
                e3 = etile[:, : 3 * Tt]
                e = [etile[:, k * Tt : (k + 1) * Tt] for k in range(3)]

                # gate: 5 tensor_scalar (4x) + 3 concatenated adds (2x)
                gt = sw.tile([128, 5 * TMAX], bf16, tag="g", name="gt")
                tu = [gt[:, u * Tt : (u + 1) * Tt] for u in range(5)]
                for u in range(5):
                    nc.vector.tensor_scalar(tu[u], x[u], w0s[u], None, Alu.mult)
                # [t0|t1] += [t2|t3]; t0 += t1; t0 += t4
                nc.vector.tensor_tensor(
                    gt[:, : 2 * Tt], gt[:, : 2 * Tt],
                    gt[:, 2 * Tt : 4 * Tt], Alu.add,
                )
                nc.vector.tensor_tensor(tu[0], tu[0], tu[1], Alu.add)
                nc.vector.tensor_tensor(tu[0], tu[0], tu[4], Alu.add)
                s_pl = tu[0]

                # coefficient-baked v copies: vpos = v*c2, vneg = -vpos
                vsc = sw.tile([128, 6 * TMAX], bf16, tag="v", name="vsc")
                v3 = ftile[:, 5 * Tt : 8 * Tt]
                vpos3 = vsc[:, : 3 * Tt]
                vneg3 = vsc[:, 3 * Tt : 6 * Tt]
                nc.vector.tensor_scalar(vpos3, v3, c2, None, Alu.mult)
                nc.vector.tensor_scalar(vneg3, v3, -c2, None, Alu.mult)
                vpos = [vsc[:, u * Tt : (u + 1) * Tt] for u in range(3)]
                vneg = [vsc[:, (3 + u) * Tt : (4 + u) * Tt] for u in range(3)]

                # k-major product planes [s_k | plus_k | minus_k]:
                # plane 3k = s'*e_k (strided out, broadcast s'),
                # 3k+1 = vpos_{k+1}*e_{k+2}, 3k+2 = vneg_{k+2}*e_{k+1}
                pr = prodp.tile([128, 9 * TMAX], bf16, tag="p", name="pr")
                pr4 = pr[:, : 9 * Tt].rearrange("p (k m t) -> p k m t", k=3, m=3)
                s_b = (
                    s_pl.rearrange("p (o t) -> p o t", o=1)
                    .broadcast_to((128, 3, Tt))
                )
                nc.vector.tensor_tensor(
                    pr4[:, :, 0, :], s_b,
                    e3.rearrange("p (k t) -> p k t", k=3), Alu.mult,
                )
                for k in range(3):
                    nc.vector.tensor_tensor(
                        pr4[:, k, 1, :], vpos[(k + 1) % 3], e[(k + 2) % 3],
                        Alu.mult,
                    )
                    nc.vector.tensor_tensor(
                        pr4[:, k, 2, :], vneg[(k + 2) % 3], e[(k + 1) % 3],
                        Alu.mult,
                    )

                # ACT: one Copy+accum per k over its contiguous 3T triple
                for k in range(3):
                    nc.scalar.activation(
                        dump_a[:, : 3 * Tt],
                        pr[:, 3 * k * Tt : 3 * (k + 1) * Tt], Act.Copy,
                        accum_out=pcol[:, t * 3 + k : t * 3 + k + 1],
                    )

            # --- fold partials: acc[:, 0:3] = sum over tiles ---
            pcol3 = pcol[:].rearrange("p (t k) -> p k t", k=3)
            nc.vector.tensor_reduce(
                out=acc[:, 0:3], in_=pcol3,
                axis=mybir.AxisListType.X, op=Alu.add,
            )

            # --- gT = transpose([g|1]): [128, 4] -> [4, 128] via PE ---
            gT_ps = psum.tile([4, 128], f32)
            nc.tensor.transpose(gT_ps[:], acc[:], identity[:])
            gT = state.tile([4, 128], f32)
            nc.scalar.copy(gT[:], gT_ps[:])

            # --- hT = relu(W1b^T(k,m) contracted with gT(k,n)) ---
            h_ps = psum.tile([128, 128], f32)
            nc.tensor.matmul(h_ps[:], lhsT=w1b_s[:], rhs=gT[:], start=True, stop=True)
            hT = state.tile([128, 128], f32)
            nc.scalar.activation(hT[:], h_ps[:], Act.Relu)

            # --- outT = W2^T . hT + b2 ---
            o_ps = psum.tile([3, 128], f32)
            nc.tensor.matmul(o_ps[:], lhsT=w2_s[:], rhs=hT[:], start=True, stop=True)
            oT = state.tile([3, 128], f32)
            nc.scalar.activation(oT[:], o_ps[:], Act.Identity, bias=b2_s[:])
            nc.sync.dma_start(out=outT, in_=oT[:])

    nc.finalize()
    return nc


def _get_nc(w_path0, w_path1):
    key = (
        np.asarray(w_path0, np.float32).tobytes(),
        np.asarray(w_path1, np.float32).tobytes(),
    )
    if _CACHED.get("key") != key:
        _CACHED["nc"] = _build(
            np.asarray(w_path0, np.float32).reshape(5),
            float(np.asarray(w_path1, np.float32).reshape(1)[0]),
        )
        _CACHED["key"] = key
    return _CACHED["nc"]


def _tile_major(shard, order):
    """[BL, N, C] -> [BL, sum_t C*Tt]: per tile, channel-planar in the
    given channel order."""
    C = len(order)
    blocks = []
    off = 0
    for Tt in TILES:
        blk = (
            shard[:, off : off + Tt, :][:, :, order]
            .transpose(0, 2, 1)
            .reshape(BL, C * Tt)
        )
        blocks.append(blk)
        off += Tt
    return np.ascontiguousarray(np.concatenate(blocks, axis=1))


def _in_maps(feats, edge_attr, W1, b1, W2, b2):
    f32 = np.float32
    W1m = np.ascontiguousarray(W1, f32).reshape(3, 128)
    b1m = np.ascontiguousarray(b1, f32).reshape(1, 128)
    W2m = np.ascontiguousarray(W2, f32).reshape(128, 3)
    b2m = np.ascontiguousarray(b2, f32).reshape(3, 1)
    feats = np.asarray(feats, f32).astype(BF16)
    edge_attr = np.asarray(edge_attr, f32).astype(BF16)
    maps = []
    for c in range(NCORES):
        sl = slice(c * BL, (c + 1) * BL)
        maps.append(
            {
                "feats": _tile_major(feats[sl], F_ORDER),
                "edge": _tile_major(edge_attr[sl], E_ORDER),
                "W1": W1m,
                "b1": b1m,
                "W2": W2m,
                "b2": b2m,
            }
        )
    return maps


def run(inputs, trace=False, tmpdir=None):
    """Run on 8 cores; returns (out [B,3], BassKernelResults)."""
    from concourse import bass_utils

    nc = _get_nc(inputs["w_path0"], inputs["w_path1"])
    maps = _in_maps(
        inputs["feats"], inputs["edge_attr"],
        inputs["W1"], inputs["b1"], inputs["W2"], inputs["b2"],
    )
    kw = {}
    if trace:
        kw.update(trace=True, tmpdir=tmpdir)
    res = bass_utils.run_bass_kernel_spmd(
        nc, maps, core_ids=list(range(NCORES)), **kw
    )
    outT_full = np.concatenate([r["outT"] for r in res.results], axis=1)  # [3, B]
    return np.ascontiguousarray(outT_full.T), res


def kernel(feats, edge_attr, w_path0, w_path1, W1, b1, W2, b2):
    out, _ = run(
        dict(
            feats=feats, edge_attr=edge_attr, w_path0=w_path0, w_path1=w_path1,
            W1=W1, b1=b1, W2=W2, b2=b2,
        )
    )
    return out

